# revision 1
# baseline (speedup 1.0000x reference)
"""Trainium2 Bass kernel for CropConLoss (supervised-contrastive style loss).

Contract: kernel(**inputs) takes the FULL unsharded inputs
(protos [64,128] f32, proj2/proj3 [4096,128] f32, target2/target3 [4096] i64)
and returns the FULL output (scalar f32 mean loss), running the compute on
8 NeuronCores via bass_utils.run_bass_kernel_spmd.

Strategy (data-parallel over the M=8192 rows of feats):
  - Each core owns 1024 query rows. The host hands every core a np.roll'd
    copy of all 8192 feature rows (its own queries first), so the
    diagonal-masking control flow is identical on every core (SPMD-safe).
  - Per core: sim tile [128 keys, 1024 q] = keysT_kt^T @ qnT via PE;
    exp via ACT with the per-key 1/(tau*|x_k|) folded into the activation
    scale (so keys never need explicit normalization); per-class sums +
    row sum accumulated with a second matmul (one-hot-augmented stationary)
    into a persistent PSUM accumulator [65+, 1024].
  - Epilogue selects numer (own-class sum + proto term) and denom
    (weighted row-sum + freq-weighted proto sums) with one-hot masks and
    ones-matmul partition reductions, then ACT Ln with fused free-dim
    accumulation; each core returns sum(loss_rows) over its 1024 rows.
  - Host sums the 8 partials and divides by 8192. No device collectives.
"""

import sys
import types

sys.path.insert(0, "/opt/trn_rl_repo")

import numpy as np

TAU = 0.1
EPS_FREQ = 1e-06
EPS_DENOM = 1e-12

N_CORES = 8
M = 8192          # total rows (2*4096)
D = 128           # feature dim
C = 64            # num classes
Q = M // N_CORES  # 1024 query rows per core
NT = M // 128     # 64 key tiles of 128


def _install_ntff_hook():
    """Shim antenv.axon_hooks (absent in this image) so trace=True works."""
    if "antenv.axon_hooks" in sys.modules:
        return
    try:
        if "/root/.axon_site" not in sys.path:
            sys.path.insert(0, "/root/.axon_site")
        import trn_agent_boot.trn_boot as tb

        hook = tb._ntff_profile_via_ctypes("/opt/axon/libaxon_pjrt.so")
        mod = types.ModuleType("antenv.axon_hooks")
        mod._hook = hook
        mod.get_axon_ntff_profile_hook = lambda: mod._hook
        mod.set_axon_ntff_profile_hook = lambda h: setattr(mod, "_hook", h)
        sys.modules["antenv.axon_hooks"] = mod
        import antenv

        antenv.axon_hooks = mod
    except Exception:
        pass


def build_nc(n_kt=NT, do_epi=True, do_main=True):
    """Build and compile the single-core Bass program (same NEFF on all 8)."""
    import concourse.bass as bass  # noqa: F401
    import concourse.mybir as mybir
    import concourse.bacc as bacc
    from concourse import tile

    f32 = mybir.dt.float32
    bf16 = mybir.dt.bfloat16
    mult = mybir.AluOpType.mult
    add = mybir.AluOpType.add
    Act = mybir.ActivationFunctionType

    nc = bacc.Bacc("TRN2", target_bir_lowering=False, debug=False,
                   num_devices=N_CORES)

    # DRAM I/O (per-core data is provided via in_maps)
    d_keysT = nc.dram_tensor("keysT", [128, M], bf16, kind="ExternalInput")
    d_keysN = nc.dram_tensor("keysN", [128, NT, 128], bf16, kind="ExternalInput")
    d_onehot = nc.dram_tensor("onehot", [128, NT, 128], bf16, kind="ExternalInput")
    d_mask = nc.dram_tensor("mask8", [128, 8, Q], bf16, kind="ExternalInput")
    d_ohqT = nc.dram_tensor("ohqT", [C + 1, Q], f32, kind="ExternalInput")
    d_fwinv = nc.dram_tensor("fwinv", [1, Q], f32, kind="ExternalInput")
    d_cfinv = nc.dram_tensor("cfinv", [C + 1, 1], f32, kind="ExternalInput")
    d_ones = nc.dram_tensor("ones65", [C + 1, 1], f32, kind="ExternalInput")
    d_ident = nc.dram_tensor("ident", [128, 128], bf16, kind="ExternalInput")
    d_protos = nc.dram_tensor("protos", [C, 128], f32, kind="ExternalInput")
    d_out = nc.dram_tensor("out", [1, 1], f32, kind="ExternalOutput")

    with tile.TileContext(nc) as tc:
        with (
            tc.tile_pool(name="const", bufs=1) as cst,
            tc.tile_pool(name="work", bufs=3) as work,
        ):
            # ---- resident SBUF tensors ----
            keysT = cst.tile([128, M], bf16, tag="keysT")
            keysN = cst.tile([128, NT, 128], bf16, tag="keysN")
            onehot = cst.tile([128, NT, 128], bf16, tag="onehot")
            mask8 = cst.tile([128, 8, Q], bf16, tag="mask8")
            ohqT = cst.tile([C + 1, Q], f32, tag="ohqT")
            fwinv = cst.tile([1, Q], f32, tag="fwinv")
            cfinv = cst.tile([C + 1, 1], f32, tag="cfinv")
            ones65 = cst.tile([C + 1, 1], f32, tag="ones65")
            ident = cst.tile([128, 128], bf16, tag="ident")
            protos = cst.tile([C, 128], f32, tag="protos")

            nc.sync.dma_start(keysN[:], d_keysN[:])
            nc.sync.dma_start(keysT[:], d_keysT[:])
            nc.sync.dma_start(onehot[:], d_onehot[:])
            nc.sync.dma_start(mask8[:], d_mask[:])
            nc.sync.dma_start(ohqT[:], d_ohqT[:])
            nc.sync.dma_start(fwinv[:], d_fwinv[:])
            nc.sync.dma_start(cfinv[:], d_cfinv[:])
            nc.sync.dma_start(ones65[:], d_ones[:])
            nc.sync.dma_start(ident[:], d_ident[:])
            nc.sync.dma_start(protos[:], d_protos[:])

            ss = cst.tile([128, NT], f32, tag="ss")       # per-key |x|^2
            srt = cst.tile([128, NT], f32, tag="srt")     # |x|
            rinv = cst.tile([128, NT], f32, tag="rinv")   # 1/|x|
            rinv10 = cst.tile([128, NT], f32, tag="rinv10")  # (1/tau)/|x|
            qnT = cst.tile([128, Q], bf16, tag="qnT")     # normalized queries, [d, q]
            protosT = cst.tile([128, C + 1], bf16, tag="protosT")
            p_t = cst.tile([C + 1, Q], f32, tag="p_t")    # exp(proto_sim/tau)

            # ---- prologue ----
            with (
                tc.tile_pool(name="pA", bufs=2, space="PSUM") as pA,
                tc.tile_pool(name="pB", bufs=1, space="PSUM") as pB,
            ):
                # per-key sum of squares -> |x| -> 1/|x|
                for rt in range(NT):
                    sq = work.tile([128, 128], f32, tag="sq")
                    nc.vector.tensor_tensor(sq[:], keysN[:, rt], keysN[:, rt],
                                            op=mult)
                    nc.vector.reduce_sum(ss[:, rt:rt + 1], sq[:],
                                         axis=mybir.AxisListType.X)
                nc.scalar.activation(srt[:], ss[:], Act.Sqrt)
                nc.vector.reciprocal(rinv[:], srt[:])
                nc.vector.tensor_scalar_mul(rinv10[:], rinv[:], 1.0 / TAU)

                # normalize own 8 query tiles, transpose into qnT [d, q]
                for t in range(8):
                    qn = work.tile([128, 128], bf16, tag="qn")
                    nc.vector.tensor_scalar_mul(qn[:], keysN[:, t],
                                                rinv[:, t:t + 1])
                    tp = pA.tile([128, 128], bf16, tag="tp")
                    nc.tensor.transpose(tp[:], qn[:], ident[:])
                    nc.vector.tensor_copy(qnT[:, t * 128:(t + 1) * 128], tp[:])

                # normalize protos, transpose into protosT cols 1..64
                psq = work.tile([C, 128], f32, tag="psq")
                ssp = work.tile([C, 1], f32, tag="ssp")
                nc.vector.tensor_tensor(psq[:], protos[:], protos[:], op=mult)
                nc.vector.reduce_sum(ssp[:], psq[:],
                                     axis=mybir.AxisListType.X)
                srtp = work.tile([C, 1], f32, tag="srtp")
                nc.scalar.activation(srtp[:], ssp[:], Act.Sqrt)
                rinvp = work.tile([C, 1], f32, tag="rinvp")
                nc.vector.reciprocal(rinvp[:], srtp[:])
                pn = work.tile([C, 128], bf16, tag="pn")
                nc.vector.tensor_scalar_mul(pn[:], protos[:], rinvp[:])
                ptp = pA.tile([128, C], bf16, tag="ptp")
                nc.tensor.transpose(ptp[:], pn[:], ident[0:C, 0:C])
                nc.vector.memset(protosT[:, 0:1], 0.0)
                nc.vector.tensor_copy(protosT[:, 1:C + 1], ptp[:])

                # proto similarities for own queries: [65, 1024]
                pp = pB.tile([C + 1, Q], f32, tag="pp")
                for j in range(Q // 512):
                    nc.tensor.matmul(pp[:, j * 512:(j + 1) * 512],
                                     protosT[:], qnT[:, j * 512:(j + 1) * 512],
                                     start=True, stop=True)
                nc.scalar.activation(p_t[:], pp[:], Act.Exp, scale=1.0 / TAU)

            # ---- main loop over 64 key tiles ----
            with tc.tile_pool(name="acc", bufs=1, space="PSUM") as acc:
                sT = acc.tile([128, Q], f32, tag="sT")
                with tc.tile_pool(name="ring", bufs=3, space="PSUM") as ring:
                    exp_tiles = {}
                    for kt in range(n_kt if do_main else 0):
                        ps = ring.tile([128, Q], f32, tag="ps")
                        for j in range(Q // 512):
                            nc.tensor.matmul(
                                ps[:, j * 512:(j + 1) * 512],
                                keysT[:, kt * 128:(kt + 1) * 128],
                                qnT[:, j * 512:(j + 1) * 512],
                                start=True, stop=True)
                        # software-pipelined: class-sum matmul for kt-1
                        if kt > 0:
                            et_p = exp_tiles.pop(kt - 1)
                            for j in range(Q // 512):
                                nc.tensor.matmul(
                                    sT[:, j * 512:(j + 1) * 512],
                                    onehot[:, kt - 1],
                                    et_p[:, j * 512:(j + 1) * 512],
                                    start=(kt - 1 == 0), stop=False)
                        et = work.tile([128, Q], bf16, tag="et")
                        nc.scalar.activation(et[:], ps[:], Act.Exp,
                                             scale=rinv10[:, kt:kt + 1])
                        if kt < 8:
                            nc.vector.tensor_tensor(et[:], et[:], mask8[:, kt],
                                                    op=mult)
                        exp_tiles[kt] = et
                    if do_main:
                        et_p = exp_tiles.pop(n_kt - 1)
                        for j in range(Q // 512):
                            nc.tensor.matmul(
                                sT[:, j * 512:(j + 1) * 512],
                                onehot[:, n_kt - 1],
                                et_p[:, j * 512:(j + 1) * 512],
                                start=(n_kt == 1), stop=True)
                    else:
                        nc.vector.memset(sT[:], 0.0)
                        zz = work.tile([128, Q], f32, tag="zz")
                        nc.vector.tensor_copy(zz[:], sT[:])

                # ---- epilogue ----
                if do_epi:
                  with tc.tile_pool(name="epi", bufs=1, space="PSUM") as epi:
                    # b[m,q] = (S_T + P_T) * onehotQ ; row0 zeroed via ohqT
                    b = cst.tile([C + 1, Q], f32, tag="b")
                    nc.vector.tensor_tensor(b[:], sT[0:C + 1, :], p_t[:], op=add)
                    nc.vector.tensor_tensor(b[:], b[:], ohqT[:], op=mult)
                    # c2[m,q] = P_T * (1/cls_freq[c]) ; row0 zeroed via cfinv
                    c2 = cst.tile([C + 1, Q], f32, tag="c2")
                    nc.vector.tensor_scalar_mul(c2[:], p_t[:], cfinv[:])

                    pn_ = epi.tile([1, Q], f32, tag="pnum")
                    pd_ = epi.tile([1, Q], f32, tag="pden")
                    for j in range(Q // 512):
                        nc.tensor.matmul(pn_[:, j * 512:(j + 1) * 512],
                                         ones65[:], b[:, j * 512:(j + 1) * 512],
                                         start=True, stop=True)
                        nc.tensor.matmul(pd_[:, j * 512:(j + 1) * 512],
                                         ones65[:], c2[:, j * 512:(j + 1) * 512],
                                         start=True, stop=True)

                    # denom = rowsum/feat_w + denom_proto + eps
                    den = cst.tile([1, Q], f32, tag="den")
                    nc.vector.tensor_tensor(den[:], sT[0:1, :], fwinv[:], op=mult)
                    nc.vector.tensor_tensor(den[:], den[:], pd_[:], op=add)
                    nc.vector.tensor_scalar_add(den[:], den[:], EPS_DENOM)

                    lbuf = cst.tile([1, Q], f32, tag="lbuf")
                    ld_s = cst.tile([1, 1], f32, tag="ld_s")
                    ln_s = cst.tile([1, 1], f32, tag="ln_s")
                    nc.scalar.activation(lbuf[:], den[:], Act.Ln,
                                         accum_out=ld_s[:])
                    lbuf2 = cst.tile([1, Q], f32, tag="lbuf2")
                    nc.scalar.activation(lbuf2[:], pn_[:], Act.Ln,
                                         accum_out=ln_s[:])
                    res = cst.tile([1, 1], f32, tag="res")
                    nc.vector.tensor_tensor(res[:], ld_s[:], ln_s[:],
                                            op=mybir.AluOpType.subtract)
                    nc.sync.dma_start(d_out[:], res[:])
                else:
                    res = cst.tile([1, 1], f32, tag="res")
                    nc.vector.tensor_copy(res[:], sT[0:1, 0:1])
                    nc.sync.dma_start(d_out[:], res[:])

    nc.compile()
    return nc


def make_in_maps(protos, proj2, target2, proj3, target3):
    import ml_dtypes

    bf16 = ml_dtypes.bfloat16
    f32 = np.float32

    feats = np.concatenate([np.asarray(proj2, dtype=f32),
                            np.asarray(proj3, dtype=f32)], axis=0)
    labels = np.concatenate([np.asarray(target2), np.asarray(target3)],
                            axis=0).astype(np.int64)

    counts = np.bincount(labels, minlength=C).astype(f32)
    cls_freq = (counts + f32(1.0)) + f32(EPS_FREQ)   # matches reference f32 math
    cfr = (f32(1.0) / cls_freq).astype(f32)

    # globals (identical on every core)
    mask = np.ones((128, 8, Q), dtype=bf16)
    k_idx = np.arange(128)
    for t in range(8):
        mask[k_idx, t, t * 128 + k_idx] = bf16(0.0)
    ident = np.eye(128, dtype=bf16)
    cfinv = np.zeros((C + 1, 1), dtype=f32)
    cfinv[1:, 0] = cfr
    ones65 = np.ones((C + 1, 1), dtype=f32)
    protos_f = np.ascontiguousarray(np.asarray(protos, dtype=f32))

    in_maps = []
    for c in range(N_CORES):
        idx = (np.arange(M) + c * Q) % M
        kf = feats[idx]                      # [8192, 128] rolled
        kl = labels[idx]

        keysT = np.ascontiguousarray(kf.T).astype(bf16)          # [128, 8192]
        keysN = np.ascontiguousarray(
            kf.reshape(NT, 128, 128).transpose(1, 0, 2)).astype(bf16)

        oh = np.zeros((M, 128), dtype=bf16)
        oh[np.arange(M), 1 + kl] = bf16(1.0)   # cols 1..64 = class indicator
        oh[:, 0] = bf16(1.0)                   # col 0 = row-sum
        onehot = np.ascontiguousarray(
            oh.reshape(NT, 128, 128).transpose(1, 0, 2))

        ohqT = np.zeros((C + 1, Q), dtype=f32)
        ohqT[1 + kl[:Q], np.arange(Q)] = f32(1.0)

        fwinv = cfr[kl[:Q]].reshape(1, Q).astype(f32)

        in_maps.append({
            "keysT": keysT,
            "keysN": keysN,
            "onehot": onehot,
            "mask8": mask,
            "ohqT": ohqT,
            "fwinv": np.ascontiguousarray(fwinv),
            "cfinv": cfinv,
            "ones65": ones65,
            "ident": ident,
            "protos": protos_f,
        })
    return in_maps


def run(in_maps, trace=False):
    _install_ntff_hook()
    from concourse import bass_utils

    nc = build_nc()
    res = bass_utils.run_bass_kernel_spmd(
        nc, in_maps, core_ids=list(range(N_CORES)), trace=trace)
    return res


def kernel(protos, proj2, target2, proj3, target3):
    in_maps = make_in_maps(protos, proj2, target2, proj3, target3)
    res = run(in_maps, trace=False)
    parts = [res.results[i]["out"][0, 0] for i in range(N_CORES)]
    total = np.sum(np.asarray(parts, dtype=np.float32))
    return np.asarray(total / np.float32(M), dtype=np.float32)



# revision 11
# speedup vs baseline: 1.1026x; 1.1026x over previous
"""Trainium2 Bass kernel for CropConLoss (supervised-contrastive style loss).

Contract: kernel(**inputs) takes the FULL unsharded inputs
(protos [64,128] f32, proj2/proj3 [4096,128] f32, target2/target3 [4096] i64)
and returns the FULL output (scalar f32 mean loss), running the compute on
8 NeuronCores via bass_utils.run_bass_kernel_spmd.

Strategy (data-parallel over the M=8192 rows of feats, ACT-roofline design):
  - Host sorts the 8192 rows by class label and l2-normalizes them (and the
    protos) in f32, so the device needs no sqrt/reciprocal and a single
    constant exp scale of 1/tau.
  - Each core owns 1024 query rows. Layout is [query-partition, key-free]:
    per q-tile (128 queries) the stationary operand is the query block of
    keysT and the 8192 keys stream through the PE in 512-col chunks.
  - exp runs on ACT over [128, 2048] PSUM chunks with accum_out producing
    the per-row key-sums for free - no second matmul, no ones-matmul for
    the row direction. ACT is the roofline engine (~64us of exp).
  - Class-sorted rows make same-class keys contiguous, so the numerator
    (own-class sum) only needs a 512-wide window around the diagonal,
    handled by two fused DVE tensor_tensor_reduce ops per q-tile with
    small host-built masks (diag-only and class-mates-minus-diag).
  - Proto terms, per-row weights, logs and the final partition reduction
    are a tiny epilogue; each core returns sum(loss_rows) over its rows.
  - Host sums the 8 partials and divides by 8192. No device collectives.
"""

import sys
import types

sys.path.insert(0, "/opt/trn_rl_repo")

import numpy as np

TAU = 0.1
EPS_FREQ = 1e-06
EPS_DENOM = 1e-12

N_CORES = 8
M = 8192          # total rows (2*4096)
D = 128           # feature dim
C = 64            # num classes
Q = M // N_CORES  # 1024 query rows per core
NQT = Q // 128    # 8 query tiles per core
CHUNK = 2048      # key chunk per ACT instruction
NCH = M // CHUNK  # 4 chunks per q-tile
W = 512           # band window width (own-class mates live here)
QOFF = 256        # own queries sit at rolled cols [QOFF, QOFF+Q)
WMARG = 192       # window starts at q-tile start - WMARG


def _install_ntff_hook():
    """Shim antenv.axon_hooks (absent in this image) so trace=True works."""
    if "antenv.axon_hooks" in sys.modules:
        return
    try:
        if "/root/.axon_site" not in sys.path:
            sys.path.insert(0, "/root/.axon_site")
        import trn_agent_boot.trn_boot as tb

        hook = tb._ntff_profile_via_ctypes("/opt/axon/libaxon_pjrt.so")
        mod = types.ModuleType("antenv.axon_hooks")
        mod._hook = hook
        mod.get_axon_ntff_profile_hook = lambda: mod._hook
        mod.set_axon_ntff_profile_hook = lambda h: setattr(mod, "_hook", h)
        sys.modules["antenv.axon_hooks"] = mod
        import antenv

        antenv.axon_hooks = mod
    except Exception:
        pass


def build_nc():
    """Build and compile the single-core Bass program (same NEFF on all 8)."""
    import concourse.bass as bass  # noqa: F401
    import concourse.mybir as mybir
    import concourse.bacc as bacc
    from concourse import tile

    f32 = mybir.dt.float32
    bf16 = mybir.dt.bfloat16
    mult = mybir.AluOpType.mult
    add = mybir.AluOpType.add
    sub = mybir.AluOpType.subtract
    Act = mybir.ActivationFunctionType

    nc = bacc.Bacc("TRN2", target_bir_lowering=False, debug=False,
                   num_devices=N_CORES)

    d_keysT = nc.dram_tensor("keysT", [128, M], bf16, kind="ExternalInput")
    d_mdiag = nc.dram_tensor("mdiag", [128, W], bf16, kind="ExternalInput")
    d_mclass = nc.dram_tensor("mclass", [128, NQT, W], bf16,
                              kind="ExternalInput")
    d_fwinv = nc.dram_tensor("fwinv", [128, NQT], f32, kind="ExternalInput")
    d_protosT = nc.dram_tensor("protosT", [128, C], bf16,
                               kind="ExternalInput")
    d_ohp = nc.dram_tensor("ohp", [128, NQT, C], f32, kind="ExternalInput")
    d_cfb = nc.dram_tensor("cfb", [128, C], f32, kind="ExternalInput")
    d_out = nc.dram_tensor("out", [1, 1], f32, kind="ExternalOutput")

    with tile.TileContext(nc) as tc:
        with (
            tc.tile_pool(name="const", bufs=1) as cst,
            tc.tile_pool(name="etring", bufs=2) as etring,
        ):
            keysT = cst.tile([128, M], bf16, tag="keysT")
            mdiag = cst.tile([128, W], bf16, tag="mdiag")
            mclass = cst.tile([128, NQT, W], bf16, tag="mclass")
            fwinv = cst.tile([128, NQT], f32, tag="fwinv")
            protosT = cst.tile([128, C], bf16, tag="protosT")
            ohp = cst.tile([128, NQT, C], f32, tag="ohp")
            cfb = cst.tile([128, C], f32, tag="cfb")

            # warm the ACT table while input DMAs stream
            warm = cst.tile([1, 1], f32, tag="warm")
            nc.vector.memset(warm[:], 0.0)
            wj = cst.tile([1, 1], f32, tag="wj")
            nc.scalar.activation(wj[:], warm[:], Act.Exp)

            nc.sync.dma_start(keysT[:], d_keysT[:])
            nc.sync.dma_start(mdiag[:], d_mdiag[:])
            nc.sync.dma_start(mclass[:], d_mclass[:])
            nc.sync.dma_start(fwinv[:], d_fwinv[:])
            nc.sync.dma_start(protosT[:], d_protosT[:])
            nc.sync.dma_start(ohp[:], d_ohp[:])
            nc.sync.dma_start(cfb[:], d_cfb[:])

            # accumulators / epilogue operands
            racc = [cst.tile([128, NQT], f32, tag=f"racc{i}",
                             name=f"racc{i}")
                    for i in range(2 * NCH)]
            dsub = cst.tile([128, NQT], f32, tag="dsub")
            own = cst.tile([128, NQT], f32, tag="own")
            nprot = cst.tile([128, NQT], f32, tag="nprot")
            dprot = cst.tile([128, NQT], f32, tag="dprot")
            etp = cst.tile([128, NQT * C], f32, tag="etp")
            junkb = cst.tile([128, CHUNK], bf16, tag="junkb")
            junkw = cst.tile([128, W], f32, tag="junkw")
            junkp = cst.tile([128, C], f32, tag="junkp")
            onescol = cst.tile([128, 1], f32, tag="onescol")
            nc.vector.memset(onescol[:], 1.0)

            # ---- proto similarities for own queries: [128q, 8*64] ----
            with tc.tile_pool(name="pp", bufs=1, space="PSUM") as ppool:
                pp = ppool.tile([128, NQT * C], f32, tag="pp")
                for t in range(NQT):
                    qc = QOFF + 128 * t
                    nc.tensor.matmul(pp[:, C * t:C * (t + 1)],
                                     keysT[:, qc:qc + 128], protosT[:],
                                     start=True, stop=True)
                nc.scalar.activation(etp[:], pp[:], Act.Exp, scale=1.0 / TAU)
            for t in range(NQT):
                sl = slice(C * t, C * (t + 1))
                nc.vector.tensor_tensor(junkp[:], etp[:, sl], ohp[:, t],
                                        op=mult)
                nc.vector.reduce_sum(nprot[:, t:t + 1], junkp[:],
                                     axis=mybir.AxisListType.X)
                nc.vector.tensor_tensor(junkp[:], etp[:, sl], cfb[:],
                                        op=mult)
                nc.vector.reduce_sum(dprot[:, t:t + 1], junkp[:],
                                     axis=mybir.AxisListType.X)

            # ---- main loop: 8 q-tiles x 4 key chunks of 2048 ----
            with tc.tile_pool(name="ring", bufs=2, space="PSUM") as ring:
                for t in range(NQT):
                    qc = QOFF + 128 * t
                    et0 = None
                    for ch in range(NCH):
                        ps = ring.tile([128, CHUNK], f32, tag="ps")
                        for j in range(CHUNK // 512):
                            cb = CHUNK * ch + 512 * j
                            nc.tensor.matmul(ps[:, 512 * j:512 * (j + 1)],
                                             keysT[:, qc:qc + 128],
                                             keysT[:, cb:cb + 512],
                                             start=True, stop=True)
                        if ch == 0:
                            et0 = etring.tile([128, CHUNK], bf16, tag="et0")
                        H = CHUNK // 2
                        for h in range(2):
                            dst = et0 if ch == 0 else junkb
                            nc.scalar.activation(
                                dst[:, H * h:H * (h + 1)],
                                ps[:, H * h:H * (h + 1)],
                                Act.Exp, scale=1.0 / TAU,
                                accum_out=racc[2 * ch + h][:, t:t + 1])
                    # band window: diag value + own-class sum on DVE
                    win = slice(64 + 128 * t, 64 + 128 * t + W)
                    nc.vector.tensor_tensor(junkw[:], et0[:, win], mdiag[:],
                                            op=mult)
                    nc.vector.reduce_sum(dsub[:, t:t + 1], junkw[:],
                                         axis=mybir.AxisListType.X)
                    nc.vector.tensor_tensor(junkw[:], et0[:, win],
                                            mclass[:, t], op=mult)
                    nc.vector.reduce_sum(own[:, t:t + 1], junkw[:],
                                         axis=mybir.AxisListType.X)

            # ---- epilogue ----
            rs = cst.tile([128, NQT], f32, tag="rs")
            nc.vector.tensor_tensor(rs[:], racc[0][:], racc[1][:], op=add)
            for i in range(2, 2 * NCH):
                nc.vector.tensor_tensor(rs[:], rs[:], racc[i][:], op=add)
            nc.vector.tensor_tensor(rs[:], rs[:], dsub[:], op=sub)

            den = cst.tile([128, NQT], f32, tag="den")
            nc.vector.tensor_tensor(den[:], rs[:], fwinv[:], op=mult)
            nc.vector.tensor_tensor(den[:], den[:], dprot[:], op=add)
            nc.vector.tensor_scalar_add(den[:], den[:], EPS_DENOM)
            num = cst.tile([128, NQT], f32, tag="num")
            nc.vector.tensor_tensor(num[:], own[:], nprot[:], op=add)

            lbuf = cst.tile([128, NQT], f32, tag="lbuf")
            l1 = cst.tile([128, 1], f32, tag="l1")
            l2 = cst.tile([128, 1], f32, tag="l2")
            nc.scalar.activation(lbuf[:], den[:], Act.Ln, accum_out=l1[:])
            nc.scalar.activation(lbuf[:], num[:], Act.Ln, accum_out=l2[:])
            diff = cst.tile([128, 1], f32, tag="diff")
            nc.vector.tensor_tensor(diff[:], l1[:], l2[:], op=sub)

            with tc.tile_pool(name="pf", bufs=1, space="PSUM") as pfp:
                pf = pfp.tile([1, 1], f32, tag="pf")
                nc.tensor.matmul(pf[:], onescol[:], diff[:],
                                 start=True, stop=True)
                res = cst.tile([1, 1], f32, tag="res")
                nc.vector.tensor_copy(res[:], pf[:])
                nc.sync.dma_start(d_out[:], res[:])

    nc.compile()
    return nc


def make_in_maps(protos, proj2, target2, proj3, target3):
    import ml_dtypes

    bf16 = ml_dtypes.bfloat16
    f32 = np.float32

    feats = np.concatenate([np.asarray(proj2, dtype=f32),
                            np.asarray(proj3, dtype=f32)], axis=0)
    labels = np.concatenate([np.asarray(target2), np.asarray(target3)],
                            axis=0).astype(np.int64)

    order = np.argsort(labels, kind="stable")
    fs = feats[order]
    ls = labels[order]
    nrm = np.sqrt((fs * fs).sum(axis=1, keepdims=True))
    fn = fs / np.maximum(nrm, f32(1e-12))

    counts = np.bincount(ls, minlength=C).astype(f32)
    # class-mates of any row must fit the [start-WMARG, end+WMARG] window
    assert counts.max() <= WMARG + 1, "class count exceeds band window"
    cls_freq = (counts + f32(1.0)) + f32(EPS_FREQ)
    cfr = (f32(1.0) / cls_freq).astype(f32)

    pr = np.asarray(protos, dtype=f32)
    pnrm = np.sqrt((pr * pr).sum(axis=1, keepdims=True))
    pn = pr / np.maximum(pnrm, f32(1e-12))
    protosT = np.ascontiguousarray(pn.T).astype(bf16)

    mdiag = np.zeros((128, W), dtype=bf16)
    mdiag[np.arange(128), np.arange(128) + WMARG] = bf16(1.0)
    cfb = np.ascontiguousarray(np.broadcast_to(cfr, (128, C))).astype(f32)

    in_maps = []
    for c in range(N_CORES):
        roll = (Q * c - QOFF) % M
        idx = (np.arange(M) + roll) % M
        kf = fn[idx]
        kl = ls[idx]

        keysT = np.ascontiguousarray(kf.T).astype(bf16)

        mclass = np.zeros((128, NQT, W), dtype=bf16)
        fwinv = np.zeros((128, NQT), dtype=f32)
        ohp = np.zeros((128, NQT, C), dtype=f32)
        for t in range(NQT):
            rows = kl[QOFF + 128 * t:QOFF + 128 * t + 128]
            win = kl[64 + 128 * t:64 + 128 * t + W]
            mc = rows[:, None] == win[None, :]
            mc[np.arange(128), np.arange(128) + WMARG] = False
            mclass[:, t, :] = mc.astype(bf16)
            fwinv[:, t] = cfr[rows]
            ohp[np.arange(128), t, rows] = f32(1.0)

        in_maps.append({
            "keysT": keysT,
            "mdiag": mdiag,
            "mclass": mclass,
            "fwinv": fwinv,
            "protosT": protosT,
            "ohp": ohp,
            "cfb": cfb,
        })
    return in_maps


def run(in_maps, trace=False):
    _install_ntff_hook()
    from concourse import bass_utils

    nc = build_nc()
    res = bass_utils.run_bass_kernel_spmd(
        nc, in_maps, core_ids=list(range(N_CORES)), trace=trace)
    return res


def kernel(protos, proj2, target2, proj3, target3):
    in_maps = make_in_maps(protos, proj2, target2, proj3, target3)
    res = run(in_maps, trace=False)
    parts = [res.results[i]["out"][0, 0] for i in range(N_CORES)]
    total = np.sum(np.asarray(parts, dtype=np.float32))
    return np.asarray(total / np.float32(M), dtype=np.float32)


# revision 13
# speedup vs baseline: 1.1044x; 1.0016x over previous
"""Trainium2 Bass kernel for CropConLoss (supervised-contrastive style loss).

Contract: kernel(**inputs) takes the FULL unsharded inputs
(protos [64,128] f32, proj2/proj3 [4096,128] f32, target2/target3 [4096] i64)
and returns the FULL output (scalar f32 mean loss), running the compute on
8 NeuronCores via bass_utils.run_bass_kernel_spmd.

Strategy (data-parallel over the M=8192 rows of feats, ACT-roofline design):
  - Host sorts the 8192 rows by class label and l2-normalizes them (and the
    protos) in f32, so the device needs no sqrt/reciprocal and a single
    constant exp scale of 1/tau.
  - Each core owns 1024 query rows. Layout is [query-partition, key-free]:
    per q-tile (128 queries) the stationary operand is the query block of
    keysT and the 8192 keys stream through the PE in 512-col chunks.
  - exp runs on ACT over [128, 2048] PSUM chunks into f32 SBUF tiles; ACT
    is the roofline engine (~64us of exp). Row-sums are per-chunk DVE
    reduces (NOT accum_out - its hidden ACTIVATION_READ_ACCUMULATOR would
    cost ~285ns of ACT time per activation).
  - Class-sorted rows make same-class keys contiguous, so the numerator
    (own-class sum) only needs a 512-wide window around the diagonal,
    handled by mask-multiply + reduce on DVE with small host-built masks.
  - Proto terms, per-row weights, logs and the final partition reduction
    are a tiny epilogue; each core returns sum(loss_rows) over its rows.
  - Host sums the 8 partials and divides by 8192. No device collectives.
"""

import sys
import types

sys.path.insert(0, "/opt/trn_rl_repo")

import numpy as np

TAU = 0.1
EPS_FREQ = 1e-06
EPS_DENOM = 1e-12

N_CORES = 8
M = 8192          # total rows (2*4096)
D = 128           # feature dim
C = 64            # num classes
Q = M // N_CORES  # 1024 query rows per core
NQT = Q // 128    # 8 query tiles per core
CHUNK = 2048      # key chunk per ACT instruction
NCH = M // CHUNK  # 4 chunks per q-tile
HK = M // 2       # keysT is split in two DMA halves
W = 512           # band window width (own-class mates live here)
QOFF = 256        # own queries sit at rolled cols [QOFF, QOFF+Q)
WMARG = 192       # window starts at q-tile start - WMARG


def _install_ntff_hook():
    """Shim antenv.axon_hooks (absent in this image) so trace=True works."""
    if "antenv.axon_hooks" in sys.modules:
        return
    try:
        if "/root/.axon_site" not in sys.path:
            sys.path.insert(0, "/root/.axon_site")
        import trn_agent_boot.trn_boot as tb

        hook = tb._ntff_profile_via_ctypes("/opt/axon/libaxon_pjrt.so")
        mod = types.ModuleType("antenv.axon_hooks")
        mod._hook = hook
        mod.get_axon_ntff_profile_hook = lambda: mod._hook
        mod.set_axon_ntff_profile_hook = lambda h: setattr(mod, "_hook", h)
        sys.modules["antenv.axon_hooks"] = mod
        import antenv

        antenv.axon_hooks = mod
    except Exception:
        pass


def build_nc():
    """Build and compile the single-core Bass program (same NEFF on all 8)."""
    import concourse.bass as bass  # noqa: F401
    import concourse.mybir as mybir
    import concourse.bacc as bacc
    from concourse import tile

    f32 = mybir.dt.float32
    bf16 = mybir.dt.bfloat16
    mult = mybir.AluOpType.mult
    add = mybir.AluOpType.add
    sub = mybir.AluOpType.subtract
    Act = mybir.ActivationFunctionType

    nc = bacc.Bacc("TRN2", target_bir_lowering=False, debug=False,
                   num_devices=N_CORES)

    d_keysT = [nc.dram_tensor(f"keysT{h}", [128, HK], bf16,
                              kind="ExternalInput") for h in range(2)]
    d_protosT = nc.dram_tensor("protosT", [128, C], bf16,
                               kind="ExternalInput")
    d_mdiag = nc.dram_tensor("mdiag", [128, W], f32, kind="ExternalInput")
    d_mclass = nc.dram_tensor("mclass", [128, NQT, W], f32,
                              kind="ExternalInput")
    d_fwinv = nc.dram_tensor("fwinv", [128, NQT], f32, kind="ExternalInput")
    d_ohp = nc.dram_tensor("ohp", [128, NQT, C], f32, kind="ExternalInput")
    d_cfb = nc.dram_tensor("cfb", [128, NQT, C], f32, kind="ExternalInput")
    d_out = nc.dram_tensor("out", [1, 1], f32, kind="ExternalOutput")

    with tile.TileContext(nc) as tc:
        with (
            tc.tile_pool(name="const", bufs=1) as cst,
            tc.tile_pool(name="etring", bufs=4) as etring,
        ):
            keysT = [cst.tile([128, HK], bf16, tag=f"keysT{h}",
                              name=f"keysT{h}") for h in range(2)]
            protosT = cst.tile([128, C], bf16, tag="protosT")
            mdiag = cst.tile([128, W], f32, tag="mdiag")
            mclass = cst.tile([128, NQT, W], f32, tag="mclass")
            fwinv = cst.tile([128, NQT], f32, tag="fwinv")
            ohp = cst.tile([128, NQT, C], f32, tag="ohp")
            cfb = cst.tile([128, NQT, C], f32, tag="cfb")

            # warm the ACT table while input DMAs stream
            warm = cst.tile([1, 1], f32, tag="warm")
            nc.vector.memset(warm[:], 0.0)
            wj = cst.tile([1, 1], f32, tag="wj")
            nc.scalar.activation(wj[:], warm[:], Act.Exp)

            nc.sync.dma_start(keysT[0][:], d_keysT[0][:])
            nc.sync.dma_start(keysT[1][:], d_keysT[1][:])
            nc.sync.dma_start(protosT[:], d_protosT[:])
            nc.sync.dma_start(mdiag[:], d_mdiag[:])
            nc.sync.dma_start(mclass[:], d_mclass[:])
            nc.sync.dma_start(fwinv[:], d_fwinv[:])
            nc.sync.dma_start(ohp[:], d_ohp[:])
            nc.sync.dma_start(cfb[:], d_cfb[:])

            # accumulators / epilogue operands
            racc = [cst.tile([128, NQT], f32, tag=f"racc{i}",
                             name=f"racc{i}")
                    for i in range(NCH)]
            dsub = cst.tile([128, NQT], f32, tag="dsub")
            own = cst.tile([128, NQT], f32, tag="own")
            nprot = cst.tile([128, NQT, 1], f32, tag="nprot")
            dprot = cst.tile([128, NQT, 1], f32, tag="dprot")
            etp = cst.tile([128, NQT, C], f32, tag="etp")
            junkp = cst.tile([128, NQT, C], f32, tag="junkp")
            junkw = cst.tile([128, W], f32, tag="junkw")
            onescol = cst.tile([128, 1], f32, tag="onescol")
            nc.vector.memset(onescol[:], 1.0)

            # ---- proto similarities for own queries: [128q, 8, 64] ----
            with tc.tile_pool(name="pp", bufs=1, space="PSUM") as ppool:
                pp = ppool.tile([128, NQT, C], f32, tag="pp")
                for t in range(NQT):
                    qc = QOFF + 128 * t
                    nc.tensor.matmul(pp[:, t], keysT[0][:, qc:qc + 128],
                                     protosT[:], start=True, stop=True)
                nc.scalar.activation(etp[:], pp[:], Act.Exp, scale=1.0 / TAU)
            # batched proto selects: one mult + one innermost-axis reduce each
            nc.vector.tensor_tensor(junkp[:], etp[:], ohp[:], op=mult)
            nc.vector.reduce_sum(nprot[:], junkp[:], axis=mybir.AxisListType.X)
            nc.vector.tensor_tensor(junkp[:], etp[:], cfb[:], op=mult)
            nc.vector.reduce_sum(dprot[:], junkp[:], axis=mybir.AxisListType.X)

            # ---- main loop: 8 q-tiles x 4 key chunks of 2048 ----
            with tc.tile_pool(name="ring", bufs=2, space="PSUM") as ring:
                for t in range(NQT):
                    qc = QOFF + 128 * t
                    for ch in range(NCH):
                        ps = ring.tile([128, CHUNK], f32, tag="ps")
                        for j in range(CHUNK // 512):
                            cb = CHUNK * ch + 512 * j
                            nc.tensor.matmul(ps[:, 512 * j:512 * (j + 1)],
                                             keysT[0][:, qc:qc + 128],
                                             keysT[cb // HK][:, cb % HK:
                                                             cb % HK + 512],
                                             start=True, stop=True)
                        et = etring.tile([128, CHUNK], f32, tag="et")
                        nc.scalar.activation(et[:], ps[:], Act.Exp,
                                             scale=1.0 / TAU)
                        if ch == 0:
                            # band window: diag value + own-class sum on DVE
                            win = slice(64 + 128 * t, 64 + 128 * t + W)
                            nc.vector.tensor_tensor(junkw[:], et[:, win],
                                                    mdiag[:], op=mult)
                            nc.vector.reduce_sum(dsub[:, t:t + 1], junkw[:],
                                                 axis=mybir.AxisListType.X)
                            nc.vector.tensor_tensor(junkw[:], et[:, win],
                                                    mclass[:, t], op=mult)
                            nc.vector.reduce_sum(own[:, t:t + 1], junkw[:],
                                                 axis=mybir.AxisListType.X)
                        nc.vector.reduce_sum(racc[ch][:, t:t + 1], et[:],
                                             axis=mybir.AxisListType.X)

            # ---- epilogue ----
            rs = cst.tile([128, NQT], f32, tag="rs")
            nc.vector.tensor_tensor(rs[:], racc[0][:], racc[1][:], op=add)
            for i in range(2, NCH):
                nc.vector.tensor_tensor(rs[:], rs[:], racc[i][:], op=add)
            nc.vector.tensor_tensor(rs[:], rs[:], dsub[:], op=sub)

            den = cst.tile([128, NQT], f32, tag="den")
            nc.vector.tensor_tensor(den[:], rs[:], fwinv[:], op=mult)
            nc.vector.tensor_tensor(den[:], den[:], dprot[:, :, 0], op=add)
            nc.vector.tensor_scalar_add(den[:], den[:], EPS_DENOM)
            num = cst.tile([128, NQT], f32, tag="num")
            nc.vector.tensor_tensor(num[:], own[:], nprot[:, :, 0], op=add)

            lbuf = cst.tile([128, NQT], f32, tag="lbuf")
            l1 = cst.tile([128, 1], f32, tag="l1")
            l2 = cst.tile([128, 1], f32, tag="l2")
            nc.scalar.activation(lbuf[:], den[:], Act.Ln, accum_out=l1[:])
            nc.scalar.activation(lbuf[:], num[:], Act.Ln, accum_out=l2[:])
            diff = cst.tile([128, 1], f32, tag="diff")
            nc.vector.tensor_tensor(diff[:], l1[:], l2[:], op=sub)

            with tc.tile_pool(name="pf", bufs=1, space="PSUM") as pfp:
                pf = pfp.tile([1, 1], f32, tag="pf")
                nc.tensor.matmul(pf[:], onescol[:], diff[:],
                                 start=True, stop=True)
                res = cst.tile([1, 1], f32, tag="res")
                nc.vector.tensor_copy(res[:], pf[:])
                nc.sync.dma_start(d_out[:], res[:])

    nc.compile()
    return nc


def make_in_maps(protos, proj2, target2, proj3, target3):
    import ml_dtypes

    bf16 = ml_dtypes.bfloat16
    f32 = np.float32

    feats = np.concatenate([np.asarray(proj2, dtype=f32),
                            np.asarray(proj3, dtype=f32)], axis=0)
    labels = np.concatenate([np.asarray(target2), np.asarray(target3)],
                            axis=0).astype(np.int64)

    order = np.argsort(labels, kind="stable")
    fs = feats[order]
    ls = labels[order]
    nrm = np.sqrt((fs * fs).sum(axis=1, keepdims=True))
    fn = fs / np.maximum(nrm, f32(1e-12))

    counts = np.bincount(ls, minlength=C).astype(f32)
    # class-mates of any row must fit the [start-WMARG, end+WMARG] window
    assert counts.max() <= WMARG + 1, "class count exceeds band window"
    cls_freq = (counts + f32(1.0)) + f32(EPS_FREQ)
    cfr = (f32(1.0) / cls_freq).astype(f32)

    pr = np.asarray(protos, dtype=f32)
    pnrm = np.sqrt((pr * pr).sum(axis=1, keepdims=True))
    pn = pr / np.maximum(pnrm, f32(1e-12))
    protosT = np.ascontiguousarray(pn.T).astype(bf16)

    mdiag = np.zeros((128, W), dtype=f32)
    mdiag[np.arange(128), np.arange(128) + WMARG] = f32(1.0)
    cfb = np.ascontiguousarray(
        np.broadcast_to(cfr, (128, NQT, C))).astype(f32)

    in_maps = []
    for c in range(N_CORES):
        roll = (Q * c - QOFF) % M
        idx = (np.arange(M) + roll) % M
        kf = fn[idx]
        kl = ls[idx]

        keysT = np.ascontiguousarray(kf.T).astype(bf16)

        mclass = np.zeros((128, NQT, W), dtype=f32)
        fwinv = np.zeros((128, NQT), dtype=f32)
        ohp = np.zeros((128, NQT, C), dtype=f32)
        for t in range(NQT):
            rows = kl[QOFF + 128 * t:QOFF + 128 * t + 128]
            win = kl[64 + 128 * t:64 + 128 * t + W]
            mc = rows[:, None] == win[None, :]
            mc[np.arange(128), np.arange(128) + WMARG] = False
            mclass[:, t, :] = mc.astype(f32)
            fwinv[:, t] = cfr[rows]
            ohp[np.arange(128), t, rows] = f32(1.0)

        in_maps.append({
            "keysT0": np.ascontiguousarray(keysT[:, :HK]),
            "keysT1": np.ascontiguousarray(keysT[:, HK:]),
            "protosT": protosT,
            "mdiag": mdiag,
            "mclass": mclass,
            "fwinv": fwinv,
            "ohp": ohp,
            "cfb": cfb,
        })
    return in_maps


def run(in_maps, trace=False):
    _install_ntff_hook()
    from concourse import bass_utils

    nc = build_nc()
    res = bass_utils.run_bass_kernel_spmd(
        nc, in_maps, core_ids=list(range(N_CORES)), trace=trace)
    return res


def kernel(protos, proj2, target2, proj3, target3):
    in_maps = make_in_maps(protos, proj2, target2, proj3, target3)
    res = run(in_maps, trace=False)
    parts = [res.results[i]["out"][0, 0] for i in range(N_CORES)]
    total = np.sum(np.asarray(parts, dtype=np.float32))
    return np.asarray(total / np.float32(M), dtype=np.float32)


# revision 16
# speedup vs baseline: 1.1516x; 1.0428x over previous
"""Trainium2 Bass kernel for CropConLoss (supervised-contrastive style loss).

Contract: kernel(**inputs) takes the FULL unsharded inputs
(protos [64,128] f32, proj2/proj3 [4096,128] f32, target2/target3 [4096] i64)
and returns the FULL output (scalar f32 mean loss), running the compute on
8 NeuronCores via bass_utils.run_bass_kernel_spmd.

Strategy (data-parallel over the M=8192 rows of feats, ACT-roofline design):
  - Host sorts the 8192 rows by class label and l2-normalizes them (and the
    protos) in f32, so the device needs no sqrt/reciprocal and a single
    constant exp scale of 1/tau.
  - Each core owns 1024 query rows. Layout is [query-partition, key-free]:
    per q-tile (128 queries) the stationary operand is the query block of
    keysT and the 8192 keys stream through the PE in 512-col chunks.
  - exp runs on ACT over [128, 2048] PSUM chunks into f32 SBUF tiles; ACT
    is the roofline engine (~64us of exp). Row-sums are per-chunk DVE
    reduces (NOT accum_out - its hidden ACTIVATION_READ_ACCUMULATOR would
    cost ~285ns of ACT time per activation).
  - Class-sorted rows make same-class keys contiguous, so the numerator
    (own-class sum) only needs a 512-wide window around the diagonal,
    handled by mask-multiply + reduce on DVE with small host-built masks.
  - Proto terms, per-row weights, logs and the final partition reduction
    are a tiny epilogue; each core returns sum(loss_rows) over its rows.
  - Host sums the 8 partials and divides by 8192. No device collectives.
"""

import sys
import types

sys.path.insert(0, "/opt/trn_rl_repo")

import numpy as np

TAU = 0.1
EPS_FREQ = 1e-06
EPS_DENOM = 1e-12

N_CORES = 8
M = 8192          # total rows (2*4096)
D = 128           # feature dim
C = 64            # num classes
Q = M // N_CORES  # 1024 query rows per core
NQT = Q // 128    # 8 query tiles per core
CHUNK = 2048      # key chunk per ACT instruction
NCH = M // CHUNK  # 4 chunks per q-tile
HK = M // 2       # keysT is split in two DMA halves
W = 512           # band window width (own-class mates live here)
QOFF = 256        # own queries sit at rolled cols [QOFF, QOFF+Q)
WMARG = 192       # window starts at q-tile start - WMARG


def _install_ntff_hook():
    """Shim antenv.axon_hooks (absent in this image) so trace=True works."""
    if "antenv.axon_hooks" in sys.modules:
        return
    try:
        if "/root/.axon_site" not in sys.path:
            sys.path.insert(0, "/root/.axon_site")
        import trn_agent_boot.trn_boot as tb

        hook = tb._ntff_profile_via_ctypes("/opt/axon/libaxon_pjrt.so")
        mod = types.ModuleType("antenv.axon_hooks")
        mod._hook = hook
        mod.get_axon_ntff_profile_hook = lambda: mod._hook
        mod.set_axon_ntff_profile_hook = lambda h: setattr(mod, "_hook", h)
        sys.modules["antenv.axon_hooks"] = mod
        import antenv

        antenv.axon_hooks = mod
    except Exception:
        pass


def build_nc():
    """Build and compile the single-core Bass program (same NEFF on all 8)."""
    import concourse.bass as bass  # noqa: F401
    import concourse.mybir as mybir
    import concourse.bacc as bacc
    from concourse import tile

    f32 = mybir.dt.float32
    bf16 = mybir.dt.bfloat16
    mult = mybir.AluOpType.mult
    add = mybir.AluOpType.add
    sub = mybir.AluOpType.subtract
    Act = mybir.ActivationFunctionType

    nc = bacc.Bacc("TRN2", target_bir_lowering=False, debug=False,
                   num_devices=N_CORES)

    d_keysT = [nc.dram_tensor(f"keysT{h}", [128, HK], bf16,
                              kind="ExternalInput") for h in range(2)]
    d_protosT = nc.dram_tensor("protosT", [128, C], bf16,
                               kind="ExternalInput")
    d_mdiag = nc.dram_tensor("mdiag", [128, W], bf16, kind="ExternalInput")
    d_mclass = nc.dram_tensor("mclass", [128, NQT, W], bf16,
                              kind="ExternalInput")
    d_fwinv = nc.dram_tensor("fwinv", [128, NQT], f32, kind="ExternalInput")
    d_ohp = nc.dram_tensor("ohp", [128, NQT, C], f32, kind="ExternalInput")
    d_cfb = nc.dram_tensor("cfb", [128, NQT, C], f32, kind="ExternalInput")
    d_out = nc.dram_tensor("out", [1, 1], f32, kind="ExternalOutput")

    with tile.TileContext(nc) as tc:
        with (
            tc.tile_pool(name="const", bufs=1) as cst,
            tc.tile_pool(name="etring", bufs=4) as etring,
        ):
            keysT = [cst.tile([128, HK], bf16, tag=f"keysT{h}",
                              name=f"keysT{h}") for h in range(2)]
            protosT = cst.tile([128, C], bf16, tag="protosT")
            mdiag = cst.tile([128, W], bf16, tag="mdiag")
            mclass = cst.tile([128, NQT, W], bf16, tag="mclass")
            fwinv = cst.tile([128, NQT], f32, tag="fwinv")
            ohp = cst.tile([128, NQT, C], f32, tag="ohp")
            cfb = cst.tile([128, NQT, C], f32, tag="cfb")

            # warm the ACT table while input DMAs stream
            warm = cst.tile([1, 1], f32, tag="warm")
            nc.vector.memset(warm[:], 0.0)
            wj = cst.tile([1, 1], f32, tag="wj")
            nc.scalar.activation(wj[:], warm[:], Act.Exp)

            nc.sync.dma_start(keysT[0][:], d_keysT[0][:])
            nc.sync.dma_start(keysT[1][:], d_keysT[1][:])
            nc.sync.dma_start(protosT[:], d_protosT[:])
            nc.sync.dma_start(mdiag[:], d_mdiag[:])
            nc.sync.dma_start(mclass[:], d_mclass[:])
            nc.sync.dma_start(fwinv[:], d_fwinv[:])
            nc.sync.dma_start(ohp[:], d_ohp[:])
            nc.sync.dma_start(cfb[:], d_cfb[:])

            # accumulators / epilogue operands
            racc = [cst.tile([128, NQT], f32, tag=f"racc{i}",
                             name=f"racc{i}")
                    for i in range(NCH)]
            dsub = cst.tile([128, NQT], f32, tag="dsub")
            own = cst.tile([128, NQT], f32, tag="own")
            nprot = cst.tile([128, NQT, 1], f32, tag="nprot")
            dprot = cst.tile([128, NQT, 1], f32, tag="dprot")
            etp = cst.tile([128, NQT, C], f32, tag="etp")
            junkp = cst.tile([128, NQT, C], f32, tag="junkp")
            junkw = cst.tile([128, W], bf16, tag="junkw")
            onescol = cst.tile([128, 1], f32, tag="onescol")
            nc.vector.memset(onescol[:], 1.0)

            # ---- proto similarities for own queries: [128q, 8, 64] ----
            with tc.tile_pool(name="pp", bufs=1, space="PSUM") as ppool:
                pp = ppool.tile([128, NQT, C], f32, tag="pp")
                for t in range(NQT):
                    qc = QOFF + 128 * t
                    nc.tensor.matmul(pp[:, t], keysT[0][:, qc:qc + 128],
                                     protosT[:], start=True, stop=True)
                nc.scalar.activation(etp[:], pp[:], Act.Exp, scale=1.0 / TAU)
            # batched proto selects: one mult + one innermost-axis reduce each
            nc.vector.tensor_tensor(junkp[:], etp[:], ohp[:], op=mult)
            nc.vector.reduce_sum(nprot[:], junkp[:], axis=mybir.AxisListType.X)
            nc.vector.tensor_tensor(junkp[:], etp[:], cfb[:], op=mult)
            nc.vector.reduce_sum(dprot[:], junkp[:], axis=mybir.AxisListType.X)

            # ---- main loop: 8 q-tiles x 4 key chunks of 2048 ----
            with tc.tile_pool(name="ring", bufs=2, space="PSUM") as ring:
                for t in range(NQT):
                    qc = QOFF + 128 * t
                    for ch in range(NCH):
                        ps = ring.tile([128, CHUNK], f32, tag="ps")
                        for j in range(CHUNK // 512):
                            cb = CHUNK * ch + 512 * j
                            nc.tensor.matmul(ps[:, 512 * j:512 * (j + 1)],
                                             keysT[0][:, qc:qc + 128],
                                             keysT[cb // HK][:, cb % HK:
                                                             cb % HK + 512],
                                             start=True, stop=True)
                        et = etring.tile([128, CHUNK], bf16, tag="et")
                        nc.scalar.activation(et[:], ps[:], Act.Exp,
                                             scale=1.0 / TAU)
                        if ch == 0:
                            # band window: diag value + own-class sum on DVE
                            win = slice(64 + 128 * t, 64 + 128 * t + W)
                            nc.vector.tensor_tensor(junkw[:], et[:, win],
                                                    mdiag[:], op=mult)
                            nc.vector.reduce_sum(dsub[:, t:t + 1], junkw[:],
                                                 axis=mybir.AxisListType.X)
                            nc.vector.tensor_tensor(junkw[:], et[:, win],
                                                    mclass[:, t], op=mult)
                            nc.vector.reduce_sum(own[:, t:t + 1], junkw[:],
                                                 axis=mybir.AxisListType.X)
                        nc.vector.reduce_sum(racc[ch][:, t:t + 1], et[:],
                                             axis=mybir.AxisListType.X)

            # ---- epilogue ----
            rs = cst.tile([128, NQT], f32, tag="rs")
            nc.vector.tensor_tensor(rs[:], racc[0][:], racc[1][:], op=add)
            for i in range(2, NCH):
                nc.vector.tensor_tensor(rs[:], rs[:], racc[i][:], op=add)
            nc.vector.tensor_tensor(rs[:], rs[:], dsub[:], op=sub)

            den = cst.tile([128, NQT], f32, tag="den")
            nc.vector.tensor_tensor(den[:], rs[:], fwinv[:], op=mult)
            nc.vector.tensor_tensor(den[:], den[:], dprot[:, :, 0], op=add)
            nc.vector.tensor_scalar_add(den[:], den[:], EPS_DENOM)
            num = cst.tile([128, NQT], f32, tag="num")
            nc.vector.tensor_tensor(num[:], own[:], nprot[:, :, 0], op=add)

            lbuf = cst.tile([128, NQT], f32, tag="lbuf")
            l1 = cst.tile([128, 1], f32, tag="l1")
            l2 = cst.tile([128, 1], f32, tag="l2")
            nc.scalar.activation(lbuf[:], den[:], Act.Ln, accum_out=l1[:])
            nc.scalar.activation(lbuf[:], num[:], Act.Ln, accum_out=l2[:])
            diff = cst.tile([128, 1], f32, tag="diff")
            nc.vector.tensor_tensor(diff[:], l1[:], l2[:], op=sub)

            with tc.tile_pool(name="pf", bufs=1, space="PSUM") as pfp:
                pf = pfp.tile([1, 1], f32, tag="pf")
                nc.tensor.matmul(pf[:], onescol[:], diff[:],
                                 start=True, stop=True)
                res = cst.tile([1, 1], f32, tag="res")
                nc.vector.tensor_copy(res[:], pf[:])
                nc.sync.dma_start(d_out[:], res[:])

    nc.compile()
    return nc


def make_in_maps(protos, proj2, target2, proj3, target3):
    import ml_dtypes

    bf16 = ml_dtypes.bfloat16
    f32 = np.float32

    feats = np.concatenate([np.asarray(proj2, dtype=f32),
                            np.asarray(proj3, dtype=f32)], axis=0)
    labels = np.concatenate([np.asarray(target2), np.asarray(target3)],
                            axis=0).astype(np.int64)

    order = np.argsort(labels, kind="stable")
    fs = feats[order]
    ls = labels[order]
    nrm = np.sqrt((fs * fs).sum(axis=1, keepdims=True))
    fn = fs / np.maximum(nrm, f32(1e-12))

    counts = np.bincount(ls, minlength=C).astype(f32)
    # class-mates of any row must fit the [start-WMARG, end+WMARG] window
    assert counts.max() <= WMARG + 1, "class count exceeds band window"
    cls_freq = (counts + f32(1.0)) + f32(EPS_FREQ)
    cfr = (f32(1.0) / cls_freq).astype(f32)

    pr = np.asarray(protos, dtype=f32)
    pnrm = np.sqrt((pr * pr).sum(axis=1, keepdims=True))
    pn = pr / np.maximum(pnrm, f32(1e-12))
    protosT = np.ascontiguousarray(pn.T).astype(bf16)

    mdiag = np.zeros((128, W), dtype=bf16)
    mdiag[np.arange(128), np.arange(128) + WMARG] = bf16(1.0)
    cfb = np.ascontiguousarray(
        np.broadcast_to(cfr, (128, NQT, C))).astype(f32)

    in_maps = []
    for c in range(N_CORES):
        roll = (Q * c - QOFF) % M
        idx = (np.arange(M) + roll) % M
        kf = fn[idx]
        kl = ls[idx]

        keysT = np.ascontiguousarray(kf.T).astype(bf16)

        mclass = np.zeros((128, NQT, W), dtype=bf16)
        fwinv = np.zeros((128, NQT), dtype=f32)
        ohp = np.zeros((128, NQT, C), dtype=f32)
        for t in range(NQT):
            rows = kl[QOFF + 128 * t:QOFF + 128 * t + 128]
            win = kl[64 + 128 * t:64 + 128 * t + W]
            mc = rows[:, None] == win[None, :]
            mc[np.arange(128), np.arange(128) + WMARG] = False
            mclass[:, t, :] = mc.astype(bf16)
            fwinv[:, t] = cfr[rows]
            ohp[np.arange(128), t, rows] = f32(1.0)

        in_maps.append({
            "keysT0": np.ascontiguousarray(keysT[:, :HK]),
            "keysT1": np.ascontiguousarray(keysT[:, HK:]),
            "protosT": protosT,
            "mdiag": mdiag,
            "mclass": mclass,
            "fwinv": fwinv,
            "ohp": ohp,
            "cfb": cfb,
        })
    return in_maps


def run(in_maps, trace=False):
    _install_ntff_hook()
    from concourse import bass_utils

    nc = build_nc()
    res = bass_utils.run_bass_kernel_spmd(
        nc, in_maps, core_ids=list(range(N_CORES)), trace=trace)
    return res


def kernel(protos, proj2, target2, proj3, target3):
    in_maps = make_in_maps(protos, proj2, target2, proj3, target3)
    res = run(in_maps, trace=False)
    parts = [res.results[i]["out"][0, 0] for i in range(N_CORES)]
    total = np.sum(np.asarray(parts, dtype=np.float32))
    return np.asarray(total / np.float32(M), dtype=np.float32)


# revision 18
# speedup vs baseline: 1.4658x; 1.2728x over previous
"""Trainium2 Bass kernel for CropConLoss (supervised-contrastive style loss).

Contract: kernel(**inputs) takes the FULL unsharded inputs
(protos [64,128] f32, proj2/proj3 [4096,128] f32, target2/target3 [4096] i64)
and returns the FULL output (scalar f32 mean loss), running the compute on
8 NeuronCores via bass_utils.run_bass_kernel_spmd.

Strategy (data-parallel over the M=8192 rows of feats, ACT-roofline design):
  - Host sorts the 8192 rows by class label and l2-normalizes them (and the
    protos) in f32, so the device needs no sqrt/reciprocal and a single
    constant exp scale of 1/tau.
  - Each core owns 1024 query rows. Layout is [query-partition, key-free]:
    per q-tile (128 queries) the stationary operand is the query block of
    keysT and the 8192 keys stream through the PE in 512-col chunks.
  - exp runs on ACT over [128, 2048] PSUM chunks into f32 SBUF tiles; ACT
    is the roofline engine (~64us of exp). Row-sums are per-chunk DVE
    reduces (NOT accum_out - its hidden ACTIVATION_READ_ACCUMULATOR would
    cost ~285ns of ACT time per activation).
  - Class-sorted rows make same-class keys contiguous, so the numerator
    (own-class sum) only needs a 512-wide window around the diagonal,
    handled by mask-multiply + reduce on DVE with small host-built masks.
  - Proto terms, per-row weights, logs and the final partition reduction
    are a tiny epilogue; each core returns sum(loss_rows) over its rows.
  - Host sums the 8 partials and divides by 8192. No device collectives.
"""

import sys
import types

sys.path.insert(0, "/opt/trn_rl_repo")

import numpy as np

TAU = 0.1
EPS_FREQ = 1e-06
EPS_DENOM = 1e-12

N_CORES = 8
M = 8192          # total rows (2*4096)
D = 128           # feature dim
C = 64            # num classes
Q = M // N_CORES  # 1024 query rows per core
NQT = Q // 128    # 8 query tiles per core
CHUNK = 2048      # key chunk per ACT instruction
NCH = M // CHUNK  # 4 chunks per q-tile
HK = M // 2       # keysT is split in two DMA halves
W = 512           # band window width (own-class mates live here)
QOFF = 256        # own queries sit at rolled cols [QOFF, QOFF+Q)
WMARG = 192       # window starts at q-tile start - WMARG


def _install_ntff_hook():
    """Shim antenv.axon_hooks (absent in this image) so trace=True works."""
    if "antenv.axon_hooks" in sys.modules:
        return
    try:
        if "/root/.axon_site" not in sys.path:
            sys.path.insert(0, "/root/.axon_site")
        import trn_agent_boot.trn_boot as tb

        hook = tb._ntff_profile_via_ctypes("/opt/axon/libaxon_pjrt.so")
        mod = types.ModuleType("antenv.axon_hooks")
        mod._hook = hook
        mod.get_axon_ntff_profile_hook = lambda: mod._hook
        mod.set_axon_ntff_profile_hook = lambda h: setattr(mod, "_hook", h)
        sys.modules["antenv.axon_hooks"] = mod
        import antenv

        antenv.axon_hooks = mod
    except Exception:
        pass


def build_nc():
    """Build and compile the single-core Bass program (same NEFF on all 8)."""
    import concourse.bass as bass  # noqa: F401
    import concourse.mybir as mybir
    import concourse.bacc as bacc
    from concourse import tile

    f32 = mybir.dt.float32
    bf16 = mybir.dt.bfloat16
    mult = mybir.AluOpType.mult
    add = mybir.AluOpType.add
    sub = mybir.AluOpType.subtract
    Act = mybir.ActivationFunctionType

    nc = bacc.Bacc("TRN2", target_bir_lowering=False, debug=False,
                   num_devices=N_CORES)

    d_keysT = [nc.dram_tensor(f"keysT{h}", [128, HK], bf16,
                              kind="ExternalInput") for h in range(2)]
    d_protosT = nc.dram_tensor("protosT", [128, C], bf16,
                               kind="ExternalInput")
    d_mdiag = nc.dram_tensor("mdiag", [128, W], bf16, kind="ExternalInput")
    d_mclass = nc.dram_tensor("mclass", [128, NQT, W], bf16,
                              kind="ExternalInput")
    d_fwinv = nc.dram_tensor("fwinv", [128, NQT], f32, kind="ExternalInput")
    d_ohp = nc.dram_tensor("ohp", [128, NQT, C], f32, kind="ExternalInput")
    d_cfb = nc.dram_tensor("cfb", [128, NQT, C], f32, kind="ExternalInput")
    d_out = nc.dram_tensor("out", [1, 1], f32, kind="ExternalOutput")

    with tile.TileContext(nc) as tc:
        with (
            tc.tile_pool(name="const", bufs=1) as cst,
            tc.tile_pool(name="etring", bufs=4) as etring,
        ):
            keysT = [cst.tile([128, HK], bf16, tag=f"keysT{h}",
                              name=f"keysT{h}") for h in range(2)]
            protosT = cst.tile([128, C], bf16, tag="protosT")
            mdiag = cst.tile([128, W], bf16, tag="mdiag")
            mclass = cst.tile([128, NQT, W], bf16, tag="mclass")
            fwinv = cst.tile([128, NQT], f32, tag="fwinv")
            ohp = cst.tile([128, NQT, C], f32, tag="ohp")
            cfb = cst.tile([128, NQT, C], f32, tag="cfb")

            # warm the ACT table while input DMAs stream
            warm = cst.tile([1, 1], f32, tag="warm")
            nc.vector.memset(warm[:], 0.0)
            wj = cst.tile([1, 1], f32, tag="wj")
            nc.scalar.activation(wj[:], warm[:], Act.Exp)

            nc.sync.dma_start(keysT[0][:], d_keysT[0][:])
            nc.sync.dma_start(keysT[1][:], d_keysT[1][:])
            nc.sync.dma_start(protosT[:], d_protosT[:])
            nc.sync.dma_start(mdiag[:], d_mdiag[:])
            nc.sync.dma_start(mclass[:], d_mclass[:])
            nc.sync.dma_start(fwinv[:], d_fwinv[:])
            nc.sync.dma_start(ohp[:], d_ohp[:])
            nc.sync.dma_start(cfb[:], d_cfb[:])

            # accumulators / epilogue operands
            racc = [cst.tile([128, NQT], f32, tag=f"racc{i}",
                             name=f"racc{i}")
                    for i in range(NCH)]
            dsub = cst.tile([128, NQT], f32, tag="dsub")
            own = cst.tile([128, NQT], f32, tag="own")
            nprot = cst.tile([128, NQT, 1], f32, tag="nprot")
            dprot = cst.tile([128, NQT, 1], f32, tag="dprot")
            etp = cst.tile([128, NQT, C], f32, tag="etp")
            junkp = cst.tile([128, NQT, C], f32, tag="junkp")
            junkw = cst.tile([128, W], bf16, tag="junkw")
            onescol = cst.tile([128, 1], f32, tag="onescol")
            nc.vector.memset(onescol[:], 1.0)

            # ---- proto similarities for own queries: [128q, 8, 64] ----
            with tc.tile_pool(name="pp", bufs=1, space="PSUM") as ppool:
                pp = ppool.tile([128, NQT, C], f32, tag="pp")
                for t in range(NQT):
                    qc = QOFF + 128 * t
                    nc.tensor.matmul(pp[:, t], keysT[0][:, qc:qc + 128],
                                     protosT[:], start=True, stop=True)
                nc.scalar.activation(etp[:], pp[:], Act.Exp, scale=1.0 / TAU)
            # batched proto selects: one mult + one innermost-axis reduce each
            nc.vector.tensor_tensor(junkp[:], etp[:], ohp[:], op=mult)
            nc.vector.reduce_sum(nprot[:], junkp[:], axis=mybir.AxisListType.X)
            nc.vector.tensor_tensor(junkp[:], etp[:], cfb[:], op=mult)
            nc.vector.reduce_sum(dprot[:], junkp[:], axis=mybir.AxisListType.X)

            # ---- main loop: 8 q-tiles x 4 key chunks of 2048 ----
            with tc.tile_pool(name="ring", bufs=2, space="PSUM") as ring:
                for t in range(NQT):
                    qc = QOFF + 128 * t
                    for ch in range(NCH):
                        ps = ring.tile([128, CHUNK], f32, tag="ps")
                        for j in range(CHUNK // 512):
                            cb = CHUNK * ch + 512 * j
                            nc.tensor.matmul(ps[:, 512 * j:512 * (j + 1)],
                                             keysT[0][:, qc:qc + 128],
                                             keysT[cb // HK][:, cb % HK:
                                                             cb % HK + 512],
                                             start=True, stop=True)
                        et = etring.tile([128, CHUNK], bf16, tag="et")
                        # split row-sum work: half via ACT accum_out
                        # (hidden RAA costs ~285ns of ACT), half via DVE
                        # reduce (2.2us each) - keeps both engines under
                        # the exp roofline
                        on_act = (NCH * t + ch) % 2 == 1
                        if on_act:
                            nc.scalar.activation(
                                et[:], ps[:], Act.Exp, scale=1.0 / TAU,
                                accum_out=racc[ch][:, t:t + 1])
                        else:
                            nc.scalar.activation(et[:], ps[:], Act.Exp,
                                                 scale=1.0 / TAU)
                        if ch == 0:
                            # band window: diag value + own-class sum on DVE
                            win = slice(64 + 128 * t, 64 + 128 * t + W)
                            nc.vector.tensor_tensor(junkw[:], et[:, win],
                                                    mdiag[:], op=mult)
                            nc.vector.reduce_sum(dsub[:, t:t + 1], junkw[:],
                                                 axis=mybir.AxisListType.X)
                            nc.vector.tensor_tensor(junkw[:], et[:, win],
                                                    mclass[:, t], op=mult)
                            nc.vector.reduce_sum(own[:, t:t + 1], junkw[:],
                                                 axis=mybir.AxisListType.X)
                        if not on_act:
                            nc.vector.reduce_sum(racc[ch][:, t:t + 1], et[:],
                                                 axis=mybir.AxisListType.X)

            # ---- epilogue ----
            rs = cst.tile([128, NQT], f32, tag="rs")
            nc.vector.tensor_tensor(rs[:], racc[0][:], racc[1][:], op=add)
            for i in range(2, NCH):
                nc.vector.tensor_tensor(rs[:], rs[:], racc[i][:], op=add)
            nc.vector.tensor_tensor(rs[:], rs[:], dsub[:], op=sub)

            den = cst.tile([128, NQT], f32, tag="den")
            nc.vector.tensor_tensor(den[:], rs[:], fwinv[:], op=mult)
            nc.vector.tensor_tensor(den[:], den[:], dprot[:, :, 0], op=add)
            nc.vector.tensor_scalar_add(den[:], den[:], EPS_DENOM)
            num = cst.tile([128, NQT], f32, tag="num")
            nc.vector.tensor_tensor(num[:], own[:], nprot[:, :, 0], op=add)

            lbuf = cst.tile([128, NQT], f32, tag="lbuf")
            l1 = cst.tile([128, 1], f32, tag="l1")
            l2 = cst.tile([128, 1], f32, tag="l2")
            nc.scalar.activation(lbuf[:], den[:], Act.Ln, accum_out=l1[:])
            nc.scalar.activation(lbuf[:], num[:], Act.Ln, accum_out=l2[:])
            diff = cst.tile([128, 1], f32, tag="diff")
            nc.vector.tensor_tensor(diff[:], l1[:], l2[:], op=sub)

            with tc.tile_pool(name="pf", bufs=1, space="PSUM") as pfp:
                pf = pfp.tile([1, 1], f32, tag="pf")
                nc.tensor.matmul(pf[:], onescol[:], diff[:],
                                 start=True, stop=True)
                res = cst.tile([1, 1], f32, tag="res")
                nc.vector.tensor_copy(res[:], pf[:])
                nc.sync.dma_start(d_out[:], res[:])

    nc.compile()
    return nc


def make_in_maps(protos, proj2, target2, proj3, target3):
    import ml_dtypes

    bf16 = ml_dtypes.bfloat16
    f32 = np.float32

    feats = np.concatenate([np.asarray(proj2, dtype=f32),
                            np.asarray(proj3, dtype=f32)], axis=0)
    labels = np.concatenate([np.asarray(target2), np.asarray(target3)],
                            axis=0).astype(np.int64)

    order = np.argsort(labels, kind="stable")
    fs = feats[order]
    ls = labels[order]
    nrm = np.sqrt((fs * fs).sum(axis=1, keepdims=True))
    fn = fs / np.maximum(nrm, f32(1e-12))

    counts = np.bincount(ls, minlength=C).astype(f32)
    # class-mates of any row must fit the [start-WMARG, end+WMARG] window
    assert counts.max() <= WMARG + 1, "class count exceeds band window"
    cls_freq = (counts + f32(1.0)) + f32(EPS_FREQ)
    cfr = (f32(1.0) / cls_freq).astype(f32)

    pr = np.asarray(protos, dtype=f32)
    pnrm = np.sqrt((pr * pr).sum(axis=1, keepdims=True))
    pn = pr / np.maximum(pnrm, f32(1e-12))
    protosT = np.ascontiguousarray(pn.T).astype(bf16)

    mdiag = np.zeros((128, W), dtype=bf16)
    mdiag[np.arange(128), np.arange(128) + WMARG] = bf16(1.0)
    cfb = np.ascontiguousarray(
        np.broadcast_to(cfr, (128, NQT, C))).astype(f32)

    in_maps = []
    for c in range(N_CORES):
        roll = (Q * c - QOFF) % M
        idx = (np.arange(M) + roll) % M
        kf = fn[idx]
        kl = ls[idx]

        keysT = np.ascontiguousarray(kf.T).astype(bf16)

        mclass = np.zeros((128, NQT, W), dtype=bf16)
        fwinv = np.zeros((128, NQT), dtype=f32)
        ohp = np.zeros((128, NQT, C), dtype=f32)
        for t in range(NQT):
            rows = kl[QOFF + 128 * t:QOFF + 128 * t + 128]
            win = kl[64 + 128 * t:64 + 128 * t + W]
            mc = rows[:, None] == win[None, :]
            mc[np.arange(128), np.arange(128) + WMARG] = False
            mclass[:, t, :] = mc.astype(bf16)
            fwinv[:, t] = cfr[rows]
            ohp[np.arange(128), t, rows] = f32(1.0)

        in_maps.append({
            "keysT0": np.ascontiguousarray(keysT[:, :HK]),
            "keysT1": np.ascontiguousarray(keysT[:, HK:]),
            "protosT": protosT,
            "mdiag": mdiag,
            "mclass": mclass,
            "fwinv": fwinv,
            "ohp": ohp,
            "cfb": cfb,
        })
    return in_maps


def run(in_maps, trace=False):
    _install_ntff_hook()
    from concourse import bass_utils

    nc = build_nc()
    res = bass_utils.run_bass_kernel_spmd(
        nc, in_maps, core_ids=list(range(N_CORES)), trace=trace)
    return res


def kernel(protos, proj2, target2, proj3, target3):
    in_maps = make_in_maps(protos, proj2, target2, proj3, target3)
    res = run(in_maps, trace=False)
    parts = [res.results[i]["out"][0, 0] for i in range(N_CORES)]
    total = np.sum(np.asarray(parts, dtype=np.float32))
    return np.asarray(total / np.float32(M), dtype=np.float32)
